# revision 27
# baseline (speedup 1.0000x reference)
"""Single-head causal self-attention on 8 Trainium2 NeuronCores (Bass/Tile).

Problem: x [1024, 256, 384], Wq/Wk/Wv [384, 64] ->
  q,k,v = x@W;  wei = softmax(mask(q k^T / sqrt(384)));  out = wei @ v
Output: [1024, 256, 64] fp32.

Strategy (data-parallel over batch, 128 batches per core, all-bf16 matmuls):
  - Host pre-transposes x to bf16 xt4[g, p, c, j] = x[4g + j//256, j%256,
    128c+p] (groups of 4 batches = 2 pairs): contraction dim C=384 on SBUF
    partitions, contiguous 6KB DMA rows, half the bytes of fp32.
  - Per batch pair (one instruction covers both batches wherever possible):
      ps_qk [128,512] = [Wk|Wq]^T x^T         (3 mm, N=512; k rows 0:64)
      k_sb/q_sb [64,512] bf16 <- DVE / ACT copies from PSUM
      v-proj reuses the drained ps_qk bank viewed [128,4,64] (x-stationary,
        12 mm, N=64 -> v lands in [t,h]); one DVE copy -> vaug [128,2,2,66]
        bf16 with persistent ones at col 64 (softmax denominator trick)
      psw  [128,1024] = weiT blocks for both batches (4 mm, K=64; batch j
        at column 512j so matmuls never straddle a PSUM bank)
      P    [128,2,384] bf16 = exp(psw/sqrt(384))  (ONE ACT op per pair)
      mask diag blocks via 4D strided AP          (ONE gpsimd mul per pair)
      pso  [128,4,65]: out[t,h]+denom: lhsT=P-block stationary, rhs=vaug
           (3 mm per batch, N=65; col 64 = softmax denominator per t)
      recip [128,4] = 1/denom; out bf16 = pso * recip (DVE broadcast mul)
  - Three-stage software pipeline (front_a p | front_b p-1 | back p-2) so
    the PE queue never drains and the copy / exp->mask chains have a full
    stage of slack.
  - Output [g, p, 8, 64] bf16 blocks; host reassembles [b, t, h] as f32.
"""

from contextlib import ExitStack

import numpy as np
import ml_dtypes

import concourse.bass as bass
import concourse.bacc as bacc
import concourse.tile as tile
from concourse import mybir
from concourse.bass_utils import run_bass_kernel_spmd

N_CORES = 8
B = 1024
T = 256
C = 384
H = 64
BPC = B // N_CORES  # 128 batches per core
NCHUNK = C // 128  # 3
NGROUP = BPC // 4  # 32 groups (2 pairs) per core
SCALE = float(C) ** -0.5

F32 = mybir.dt.float32
BF16 = mybir.dt.bfloat16
BF = ml_dtypes.bfloat16


def build_nc(bpc: int = BPC):
    npair = bpc // 2
    ngroup = bpc // 4
    nc = bacc.Bacc(
        "TRN2", target_bir_lowering=False, debug=False, num_devices=N_CORES
    )

    xt4 = nc.dram_tensor("xt4", [ngroup, 128, NCHUNK, 1024], BF16, kind="ExternalInput").ap()
    wkq = nc.dram_tensor("wkq", [128, NCHUNK, 128], BF16, kind="ExternalInput").ap()
    wv = nc.dram_tensor("wv", [128, NCHUNK, H], BF16, kind="ExternalInput").ap()
    mask = nc.dram_tensor("mask", [128, 2, 2, 128], BF16, kind="ExternalInput").ap()
    outF = nc.dram_tensor("outF", [ngroup, 128, 8, H], BF16, kind="ExternalOutput").ap()

    with ExitStack() as ctx:
        tc = ctx.enter_context(tile.TileContext(nc))

        const = ctx.enter_context(tc.tile_pool(name="const", bufs=1))
        wkq_sb = const.tile([128, NCHUNK, 128], BF16, tag="wkq")
        nc.sync.dma_start(wkq_sb[:], wkq)
        wv_sb = const.tile([128, NCHUNK, H], BF16, tag="wv")
        nc.sync.dma_start(wv_sb[:], wv)
        mask_sb = const.tile([128, 2, 2, 128], BF16, tag="mask")
        nc.sync.dma_start(mask_sb[:], mask)

        # Persistent v_aug pair-tiles [batch, s-half, 66]: v at [:, j, i, 0:64],
        # ones at col 64. 4 slots cover the pipeline depth.
        NSLOT = 4
        vaug = []
        for i in range(NSLOT):
            v_t = const.tile([128, 2, 2, 66], BF16, tag=f"vaug{i}")
            nc.gpsimd.memset(v_t[:, :, :, 64:65], 1.0)
            vaug.append(v_t)

        xt_pool = ctx.enter_context(tc.tile_pool(name="xt", bufs=3))
        qk_pool = ctx.enter_context(tc.tile_pool(name="qk", bufs=3))
        p_pool = ctx.enter_context(tc.tile_pool(name="pp", bufs=3))
        o_pool = ctx.enter_context(tc.tile_pool(name="oo", bufs=2))
        r_pool = ctx.enter_context(tc.tile_pool(name="rr", bufs=2))
        psqk_pool = ctx.enter_context(tc.tile_pool(name="psqk", bufs=2, space="PSUM"))
        psv_pool = ctx.enter_context(tc.tile_pool(name="psv", bufs=2, space="PSUM"))
        psw_pool = ctx.enter_context(tc.tile_pool(name="psw", bufs=1, space="PSUM"))
        pso_pool = ctx.enter_context(tc.tile_pool(name="pso", bufs=2, space="PSUM"))

        xt_tiles = {}
        st_a = {}  # pair -> (xt, base, ps_qk, q_sb, k_sb) from front_a
        st_b = {}  # pair -> (P, vaug slot) from front_b
        st_o = {}  # group -> out_sb

        def front_a(p):
            """DMA + qk projection + q/k copies for pair p."""
            g, r = divmod(p, 2)
            if r == 0:
                xt = xt_pool.tile([128, NCHUNK, 1024], BF16, tag="xt")
                nc.sync.dma_start(xt[:], xt4[g])
                xt_tiles[g] = xt
            xt = xt_tiles[g]
            base = 512 * r

            ps_qk = psqk_pool.tile([128, 512], F32, tag="psqk")
            for c in range(NCHUNK):
                nc.tensor.matmul(
                    ps_qk[:],
                    lhsT=wkq_sb[:, c, :],
                    rhs=xt[:, c, base : base + 512],
                    start=(c == 0),
                    stop=(c == NCHUNK - 1),
                )

            k_sb = qk_pool.tile([H, 512], BF16, tag="k")
            nc.vector.tensor_copy(k_sb[:], ps_qk[0:H, :])
            q_sb = qk_pool.tile([H, 512], BF16, tag="q")
            nc.scalar.copy(q_sb[:], ps_qk[H:128, :])

            ps_v = psv_pool.tile([128, 4, H], F32, tag="psv")
            for blk in range(4):
                for c in range(NCHUNK):
                    nc.tensor.matmul(
                        ps_v[:, blk, :],
                        lhsT=xt[:, c, base + 128 * blk : base + 128 * (blk + 1)],
                        rhs=wv_sb[:, c, :],
                        start=(c == 0),
                        stop=(c == NCHUNK - 1),
                    )
            slot = vaug[p % NSLOT]
            nc.scalar.copy(slot[:, 0, :, 0:64], ps_v[:, 0:2, :])
            nc.vector.tensor_copy(slot[:, 1, :, 0:64], ps_v[:, 2:4, :])
            st_a[p] = (q_sb, k_sb, slot)

        def front_b(p):
            """wei matmuls + exp + mask for pair p."""
            q_sb, k_sb, slot = st_a.pop(p)

            psw = psw_pool.tile([128, 1024], F32, tag="psw")
            for j in range(2):
                for s in range(2):
                    nc.tensor.matmul(
                        psw[:, 512 * j + 128 * s : 512 * j + 128 * s + 128],
                        lhsT=k_sb[:, 256 * j : 256 * j + 128],
                        rhs=q_sb[:, 256 * j + 128 * s : 256 * j + 128 * s + 128],
                        start=True,
                        stop=True,
                    )
                nc.tensor.matmul(
                    psw[:, 512 * j + 256 : 512 * j + 384],
                    lhsT=k_sb[:, 256 * j + 128 : 256 * j + 256],
                    rhs=q_sb[:, 256 * j + 128 : 256 * j + 256],
                    start=True,
                    stop=True,
                )

            P = p_pool.tile([128, 2, 3, 128], BF16, tag="p")
            nc.scalar.activation(
                P[:].rearrange("p j a b -> p (j a b)").rearrange("p (j x) -> p j x", j=2),
                psw[:].rearrange("p (j x) -> p j x", j=2)[:, :, 0:384],
                mybir.ActivationFunctionType.Exp,
                scale=SCALE,
            )
            nc.vector.tensor_mul(P[:, :, 0::2, :], P[:, :, 0::2, :], mask_sb[:])
            st_b[p] = (P, slot)

        def back(p):
            """Out matmuls + normalize + output DMA for pair p."""
            g, r = divmod(p, 2)
            P, slot = st_b.pop(p)
            pso = pso_pool.tile([128, 4, 65], F32, tag="pso")
            for j in range(2):
                nc.tensor.matmul(
                    pso[:, 2 * j, :],
                    lhsT=P[:, j, 0, :],
                    rhs=slot[:, j, 0, 0:65],
                    start=True,
                    stop=True,
                )
                nc.tensor.matmul(
                    pso[:, 2 * j + 1, :],
                    lhsT=P[:, j, 1, :],
                    rhs=slot[:, j, 0, 0:65],
                    start=True,
                    stop=False,
                )
                nc.tensor.matmul(
                    pso[:, 2 * j + 1, :],
                    lhsT=P[:, j, 2, :],
                    rhs=slot[:, j, 1, 0:65],
                    start=False,
                    stop=True,
                )

            recip = r_pool.tile([128, 4], F32, tag="recip")
            nc.vector.reciprocal(recip[:], pso[:, :, 64])

            if r == 0:
                out_sb = o_pool.tile([128, 8, H], BF16, tag="out")
                st_o[g] = out_sb
            else:
                out_sb = st_o[g]
            nc.vector.tensor_mul(
                out_sb[:, 4 * r : 4 * r + 4, :],
                pso[:, :, 0:64],
                recip[:].unsqueeze(-1).broadcast_to([128, 4, H]),
            )
            if r == 1:
                nc.sync.dma_start(outF[g], st_o.pop(g)[:])

        # Three-stage software pipeline.
        front_a(0)
        front_a(1)
        front_b(0)
        for p in range(2, npair):
            front_a(p)
            front_b(p - 1)
            back(p - 2)
        front_b(npair - 1)
        back(npair - 2)
        back(npair - 1)

    nc.finalize()
    return nc


def _host_inputs(x, Wq, Wk, Wv):
    B_, T_, C_ = x.shape
    assert (B_, T_, C_) == (B, T, C), (B_, T_, C_)
    # xt4[g, p, c, j] = x[4g + j//256, j%256, 128c + p], bf16
    xh = np.ascontiguousarray(
        x.reshape(B // 4, 4, T, NCHUNK, 128).transpose(0, 4, 3, 1, 2)
        .reshape(B // 4, 128, NCHUNK, 4 * T)
    ).astype(BF)
    wkq_h = np.ascontiguousarray(
        np.concatenate([Wk, Wq], axis=1).reshape(NCHUNK, 128, 128).transpose(1, 0, 2)
    ).astype(BF)
    wv_h = np.ascontiguousarray(
        Wv.reshape(NCHUNK, 128, H).transpose(1, 0, 2)
    ).astype(BF)
    mask1 = np.triu(np.ones((128, 128), dtype=np.float32))
    mask_h = np.ascontiguousarray(
        np.broadcast_to(mask1[:, None, None, :], (128, 2, 2, 128)).copy()
    ).astype(BF)
    return xh, wkq_h, wv_h, mask_h


def _host_output(res, bpc=BPC):
    # outF [ngroup, 128, 8, 64] bf16: block 2j+k = batch j of group, t-half k.
    outs = []
    for i in range(N_CORES):
        a = np.asarray(res.results[i]["outF"]).astype(np.float32)
        a = a.reshape(bpc // 4, 128, 4, 2, H).transpose(0, 2, 3, 1, 4)
        outs.append(a.reshape(bpc, T, H))
    return np.ascontiguousarray(np.concatenate(outs, axis=0))


def kernel(x, Wq, Wk, Wv):
    x = np.asarray(x, dtype=np.float32)
    Wq = np.asarray(Wq, dtype=np.float32)
    Wk = np.asarray(Wk, dtype=np.float32)
    Wv = np.asarray(Wv, dtype=np.float32)

    xh, wkq_h, wv_h, mask_h = _host_inputs(x, Wq, Wk, Wv)

    nc = build_nc(BPC)
    in_maps = [
        {
            "xt4": xh[i * NGROUP : (i + 1) * NGROUP],
            "wkq": wkq_h,
            "wv": wv_h,
            "mask": mask_h,
        }
        for i in range(N_CORES)
    ]
    res = run_bass_kernel_spmd(nc, in_maps, list(range(N_CORES)))
    return _host_output(res, BPC)


# revision 33
# speedup vs baseline: 1.0644x; 1.0644x over previous
"""Single-head causal self-attention on 8 Trainium2 NeuronCores (Bass/Tile).

Problem: x [1024, 256, 384], Wq/Wk/Wv [384, 64] ->
  q,k,v = x@W;  wei = softmax(mask(q k^T / sqrt(384)));  out = wei @ v
Output: [1024, 256, 64] fp32.

Strategy (data-parallel over batch, 128 batches per core, all-bf16 matmuls):
  - Host pre-transposes x to bf16 xt4[g, p, c, j] = x[4g + j//256, j%256,
    128c+p] (groups of 4 batches = 2 pairs): contraction dim C=384 on SBUF
    partitions, contiguous 6KB DMA rows, half the bytes of fp32.
  - Per batch pair (one instruction covers both batches wherever possible):
      ps_qk [128,512] = [Wk|Wq]^T x^T         (3 mm, N=512; k rows 0:64)
      k_sb/q_sb [64,512] bf16 <- DVE / ACT copies from PSUM
      v-proj reuses the drained ps_qk bank viewed [128,4,64] (x-stationary,
        12 mm, N=64 -> v lands in [t,h]); one DVE copy -> vaug [128,2,2,66]
        bf16 with persistent ones at col 64 (softmax denominator trick)
      psw  [128,1024] = weiT blocks for both batches (4 mm, K=64; batch j
        at column 512j so matmuls never straddle a PSUM bank)
      P    [128,2,384] bf16 = exp(psw/sqrt(384))  (ONE ACT op per pair)
      mask diag blocks via 4D strided AP          (ONE gpsimd mul per pair)
      pso  [128,4,65]: out[t,h]+denom: lhsT=P-block stationary, rhs=vaug
           (3 mm per batch, N=65; col 64 = softmax denominator per t)
      recip [128,4] = 1/denom; out bf16 = pso * recip (DVE broadcast mul)
  - Three-stage software pipeline (front_a p | front_b p-1 | back p-2) so
    the PE queue never drains and the copy / exp->mask chains have a full
    stage of slack.
  - Output [g, p, 8, 64] bf16 blocks; host reassembles [b, t, h] as f32.
"""

from contextlib import ExitStack

import numpy as np
import ml_dtypes

import concourse.bass as bass
import concourse.bacc as bacc
import concourse.tile as tile
from concourse import mybir
from concourse.bass_utils import run_bass_kernel_spmd

N_CORES = 8
B = 1024
T = 256
C = 384
H = 64
BPC = B // N_CORES  # 128 batches per core
NCHUNK = C // 128  # 3
NGROUP = BPC // 4  # 32 groups (2 pairs) per core
SCALE = float(C) ** -0.5

F32 = mybir.dt.float32
BF16 = mybir.dt.bfloat16
BF = ml_dtypes.bfloat16


def build_nc(bpc: int = BPC):
    npair = bpc // 2
    ngroup = bpc // 4
    nc = bacc.Bacc(
        "TRN2", target_bir_lowering=False, debug=False, num_devices=N_CORES
    )

    xt4 = nc.dram_tensor("xt4", [ngroup, 128, NCHUNK, 1024], BF16, kind="ExternalInput").ap()
    wkq = nc.dram_tensor("wkq", [128, NCHUNK, 128], BF16, kind="ExternalInput").ap()
    wv = nc.dram_tensor("wv", [128, NCHUNK, H], BF16, kind="ExternalInput").ap()
    mask = nc.dram_tensor("mask", [128, 128], BF16, kind="ExternalInput").ap()
    outF = nc.dram_tensor("outF", [ngroup, 128, 8, H], BF16, kind="ExternalOutput").ap()

    with ExitStack() as ctx:
        tc = ctx.enter_context(tile.TileContext(nc))

        const = ctx.enter_context(tc.tile_pool(name="const", bufs=1))
        # First group's input goes out ahead of the constants so the PE's
        # initial matmul wait is as short as possible (chunk 0 lands first).
        xt_pool = ctx.enter_context(tc.tile_pool(name="xt", bufs=3))
        xt0 = xt_pool.tile([128, NCHUNK, 1024], BF16, tag="xt")
        for c in range(NCHUNK):
            nc.sync.dma_start(xt0[:, c, :], xt4[0][:, c, :])

        wkq_sb = const.tile([128, NCHUNK, 128], BF16, tag="wkq")
        nc.sync.dma_start(wkq_sb[:], wkq)
        wv_sb = const.tile([128, NCHUNK, H], BF16, tag="wv")
        nc.sync.dma_start(wv_sb[:], wv)
        mask_sb = const.tile([128, 128], BF16, tag="mask")
        nc.sync.dma_start(mask_sb[:], mask)
        mask_bc = mask_sb[:].unsqueeze(1).unsqueeze(1).broadcast_to([128, 2, 2, 128])

        # Persistent v_aug pair-tiles [batch, s-half, 66]: v at [:, j, i, 0:64],
        # ones at col 64. 4 slots cover the pipeline depth.
        NSLOT = 4
        vaug = []
        for i in range(NSLOT):
            v_t = const.tile([128, 2, 2, 66], BF16, tag=f"vaug{i}")
            nc.gpsimd.memset(v_t[:, :, :, 64:65], 1.0)
            vaug.append(v_t)

        qk_pool = ctx.enter_context(tc.tile_pool(name="qk", bufs=3))
        p_pool = ctx.enter_context(tc.tile_pool(name="pp", bufs=3))
        o_pool = ctx.enter_context(tc.tile_pool(name="oo", bufs=2))
        r_pool = ctx.enter_context(tc.tile_pool(name="rr", bufs=2))
        psqk_pool = ctx.enter_context(tc.tile_pool(name="psqk", bufs=2, space="PSUM"))
        psv_pool = ctx.enter_context(tc.tile_pool(name="psv", bufs=2, space="PSUM"))
        psw_pool = ctx.enter_context(tc.tile_pool(name="psw", bufs=1, space="PSUM"))
        pso_pool = ctx.enter_context(tc.tile_pool(name="pso", bufs=2, space="PSUM"))

        xt_tiles = {}
        st_a = {}  # pair -> (xt, base, ps_qk, q_sb, k_sb) from front_a
        st_b = {}  # pair -> (P, vaug slot) from front_b
        st_o = {}  # group -> out_sb

        def front_a(p):
            """DMA + qk projection + q/k copies for pair p."""
            g, r = divmod(p, 2)
            if r == 0:
                if g == 0:
                    xt_tiles[g] = xt0
                else:
                    xt = xt_pool.tile([128, NCHUNK, 1024], BF16, tag="xt")
                    nc.sync.dma_start(xt[:], xt4[g])
                    xt_tiles[g] = xt
            xt = xt_tiles[g]
            base = 512 * r

            ps_qk = psqk_pool.tile([128, 512], F32, tag="psqk")
            for c in range(NCHUNK):
                nc.tensor.matmul(
                    ps_qk[:],
                    lhsT=wkq_sb[:, c, :],
                    rhs=xt[:, c, base : base + 512],
                    start=(c == 0),
                    stop=(c == NCHUNK - 1),
                )

            k_sb = qk_pool.tile([H, 512], BF16, tag="k")
            nc.vector.tensor_copy(k_sb[:], ps_qk[0:H, :])
            q_sb = qk_pool.tile([H, 512], BF16, tag="q")
            nc.scalar.copy(q_sb[:], ps_qk[H:128, :])

            ps_v = psv_pool.tile([128, 4, H], F32, tag="psv")
            for blk in range(4):
                for c in range(NCHUNK):
                    nc.tensor.matmul(
                        ps_v[:, blk, :],
                        lhsT=xt[:, c, base + 128 * blk : base + 128 * (blk + 1)],
                        rhs=wv_sb[:, c, :],
                        start=(c == 0),
                        stop=(c == NCHUNK - 1),
                    )
            slot = vaug[p % NSLOT]
            nc.scalar.copy(slot[:, 0, :, 0:64], ps_v[:, 0:2, :])
            nc.vector.tensor_copy(slot[:, 1, :, 0:64], ps_v[:, 2:4, :])
            st_a[p] = (q_sb, k_sb, slot)

        def front_b(p):
            """wei matmuls + exp + mask for pair p."""
            q_sb, k_sb, slot = st_a.pop(p)

            psw = psw_pool.tile([128, 1024], F32, tag="psw")
            for j in range(2):
                nc.tensor.matmul(
                    psw[:, 512 * j : 512 * j + 256],
                    lhsT=k_sb[:, 256 * j : 256 * j + 128],
                    rhs=q_sb[:, 256 * j : 256 * j + 256],
                    start=True,
                    stop=True,
                )
                nc.tensor.matmul(
                    psw[:, 512 * j + 256 : 512 * j + 384],
                    lhsT=k_sb[:, 256 * j + 128 : 256 * j + 256],
                    rhs=q_sb[:, 256 * j + 128 : 256 * j + 256],
                    start=True,
                    stop=True,
                )

            P = p_pool.tile([128, 2, 3, 128], BF16, tag="p")
            nc.scalar.activation(
                P[:].rearrange("p j a b -> p (j a b)").rearrange("p (j x) -> p j x", j=2),
                psw[:].rearrange("p (j x) -> p j x", j=2)[:, :, 0:384],
                mybir.ActivationFunctionType.Exp,
                scale=SCALE,
            )
            nc.vector.tensor_mul(P[:, :, 0::2, :], P[:, :, 0::2, :], mask_bc)
            st_b[p] = (P, slot)

        def back(p):
            """Out matmuls + normalize + output DMA for pair p."""
            g, r = divmod(p, 2)
            P, slot = st_b.pop(p)
            pso = pso_pool.tile([128, 4, 65], F32, tag="pso")
            for j in range(2):
                nc.tensor.matmul(
                    pso[:, 2 * j, :],
                    lhsT=P[:, j, 0, :],
                    rhs=slot[:, j, 0, 0:65],
                    start=True,
                    stop=True,
                )
                nc.tensor.matmul(
                    pso[:, 2 * j + 1, :],
                    lhsT=P[:, j, 1, :],
                    rhs=slot[:, j, 0, 0:65],
                    start=True,
                    stop=False,
                )
                nc.tensor.matmul(
                    pso[:, 2 * j + 1, :],
                    lhsT=P[:, j, 2, :],
                    rhs=slot[:, j, 1, 0:65],
                    start=False,
                    stop=True,
                )

            recip = r_pool.tile([128, 4], F32, tag="recip")
            nc.vector.reciprocal(recip[:], pso[:, :, 64])

            if r == 0:
                out_sb = o_pool.tile([128, 8, H], BF16, tag="out")
                st_o[g] = out_sb
            else:
                out_sb = st_o[g]
            nc.vector.tensor_mul(
                out_sb[:, 4 * r : 4 * r + 4, :],
                pso[:, :, 0:64],
                recip[:].unsqueeze(-1).broadcast_to([128, 4, H]),
            )
            if r == 1:
                nc.sync.dma_start(outF[g], st_o.pop(g)[:])

        # Three-stage software pipeline.
        front_a(0)
        front_a(1)
        front_b(0)
        for p in range(2, npair):
            front_a(p)
            front_b(p - 1)
            back(p - 2)
        front_b(npair - 1)
        back(npair - 2)
        back(npair - 1)

    nc.finalize()
    return nc


def _host_inputs(x, Wq, Wk, Wv):
    B_, T_, C_ = x.shape
    assert (B_, T_, C_) == (B, T, C), (B_, T_, C_)
    # xt4[g, p, c, j] = x[4g + j//256, j%256, 128c + p], bf16
    xh = np.ascontiguousarray(
        x.reshape(B // 4, 4, T, NCHUNK, 128).transpose(0, 4, 3, 1, 2)
        .reshape(B // 4, 128, NCHUNK, 4 * T)
    ).astype(BF)
    wkq_h = np.ascontiguousarray(
        np.concatenate([Wk, Wq], axis=1).reshape(NCHUNK, 128, 128).transpose(1, 0, 2)
    ).astype(BF)
    wv_h = np.ascontiguousarray(
        Wv.reshape(NCHUNK, 128, H).transpose(1, 0, 2)
    ).astype(BF)
    mask_h = np.triu(np.ones((128, 128), dtype=np.float32)).astype(BF)
    return xh, wkq_h, wv_h, mask_h


def _host_output(res, bpc=BPC):
    # outF [ngroup, 128, 8, 64] bf16: block 2j+k = batch j of group, t-half k.
    outs = []
    for i in range(N_CORES):
        a = np.asarray(res.results[i]["outF"]).astype(np.float32)
        a = a.reshape(bpc // 4, 128, 4, 2, H).transpose(0, 2, 3, 1, 4)
        outs.append(a.reshape(bpc, T, H))
    return np.ascontiguousarray(np.concatenate(outs, axis=0))


def kernel(x, Wq, Wk, Wv):
    x = np.asarray(x, dtype=np.float32)
    Wq = np.asarray(Wq, dtype=np.float32)
    Wk = np.asarray(Wk, dtype=np.float32)
    Wv = np.asarray(Wv, dtype=np.float32)

    xh, wkq_h, wv_h, mask_h = _host_inputs(x, Wq, Wk, Wv)

    nc = build_nc(BPC)
    in_maps = [
        {
            "xt4": xh[i * NGROUP : (i + 1) * NGROUP],
            "wkq": wkq_h,
            "wv": wv_h,
            "mask": mask_h,
        }
        for i in range(N_CORES)
    ]
    res = run_bass_kernel_spmd(nc, in_maps, list(range(N_CORES)))
    return _host_output(res, BPC)


# revision 34
# speedup vs baseline: 1.0762x; 1.0111x over previous
"""Single-head causal self-attention on 8 Trainium2 NeuronCores (Bass/Tile).

Problem: x [1024, 256, 384], Wq/Wk/Wv [384, 64] ->
  q,k,v = x@W;  wei = softmax(mask(q k^T / sqrt(384)));  out = wei @ v
Output: [1024, 256, 64] fp32.

Strategy (data-parallel over batch, 128 batches per core, all-bf16 matmuls):
  - Host pre-transposes x to bf16 xt4[g, p, c, j] = x[4g + j//256, j%256,
    128c+p] (groups of 4 batches = 2 pairs): contraction dim C=384 on SBUF
    partitions, contiguous 6KB DMA rows, half the bytes of fp32.
  - Per batch pair (one instruction covers both batches wherever possible):
      ps_qk [128,512] = [Wk|Wq]^T x^T         (3 mm, N=512; k rows 0:64)
      k_sb/q_sb [64,512] bf16 <- DVE / ACT copies from PSUM
      v-proj reuses the drained ps_qk bank viewed [128,4,64] (x-stationary,
        12 mm, N=64 -> v lands in [t,h]); one DVE copy -> vaug [128,2,2,66]
        bf16 with persistent ones at col 64 (softmax denominator trick)
      psw  [128,1024] = weiT blocks for both batches (4 mm, K=64; batch j
        at column 512j so matmuls never straddle a PSUM bank)
      P    [128,2,384] bf16 = exp(psw/sqrt(384))  (ONE ACT op per pair)
      mask diag blocks via 4D strided AP          (ONE gpsimd mul per pair)
      pso  [128,4,65]: out[t,h]+denom: lhsT=P-block stationary, rhs=vaug
           (3 mm per batch, N=65; col 64 = softmax denominator per t)
      recip [128,4] = 1/denom; out bf16 = pso * recip (DVE broadcast mul)
  - Three-stage software pipeline (front_a p | front_b p-1 | back p-2) so
    the PE queue never drains and the copy / exp->mask chains have a full
    stage of slack.
  - Output [g, p, 8, 64] bf16 blocks; host reassembles [b, t, h] as f32.
"""

from contextlib import ExitStack

import numpy as np
import ml_dtypes

import concourse.bass as bass
import concourse.bacc as bacc
import concourse.tile as tile
from concourse import mybir
from concourse.bass_utils import run_bass_kernel_spmd

N_CORES = 8
B = 1024
T = 256
C = 384
H = 64
BPC = B // N_CORES  # 128 batches per core
NCHUNK = C // 128  # 3
NGROUP = BPC // 4  # 32 groups (2 pairs) per core
SCALE = float(C) ** -0.5

F32 = mybir.dt.float32
BF16 = mybir.dt.bfloat16
BF = ml_dtypes.bfloat16


def build_nc(bpc: int = BPC):
    npair = bpc // 2
    ngroup = bpc // 4
    nc = bacc.Bacc(
        "TRN2", target_bir_lowering=False, debug=False, num_devices=N_CORES
    )

    xt4 = nc.dram_tensor("xt4", [ngroup, 128, NCHUNK, 1024], BF16, kind="ExternalInput").ap()
    wkq = nc.dram_tensor("wkq", [128, NCHUNK, 128], BF16, kind="ExternalInput").ap()
    wv = nc.dram_tensor("wv", [128, NCHUNK, H], BF16, kind="ExternalInput").ap()
    mask = nc.dram_tensor("mask", [128, 128], BF16, kind="ExternalInput").ap()
    outF = nc.dram_tensor("outF", [ngroup, 128, 8, H], BF16, kind="ExternalOutput").ap()

    with ExitStack() as ctx:
        tc = ctx.enter_context(tile.TileContext(nc))

        const = ctx.enter_context(tc.tile_pool(name="const", bufs=1))
        # First group's input goes out ahead of the constants so the PE's
        # initial matmul wait is as short as possible (chunk 0 lands first).
        xt_pool = ctx.enter_context(tc.tile_pool(name="xt", bufs=4))
        xt0 = xt_pool.tile([128, NCHUNK, 1024], BF16, tag="xt")
        nc.sync.dma_start(xt0[:, 0, 0:512], xt4[0][:, 0, 0:512])
        wkq_sb = const.tile([128, NCHUNK, 128], BF16, tag="wkq")
        nc.sync.dma_start(wkq_sb[:], wkq)
        nc.sync.dma_start(xt0[:, 1, 0:512], xt4[0][:, 1, 0:512])
        nc.sync.dma_start(xt0[:, 2, 0:512], xt4[0][:, 2, 0:512])
        for c in range(NCHUNK):
            nc.sync.dma_start(xt0[:, c, 512:1024], xt4[0][:, c, 512:1024])
        wv_sb = const.tile([128, NCHUNK, H], BF16, tag="wv")
        nc.sync.dma_start(wv_sb[:], wv)
        mask_sb = const.tile([128, 128], BF16, tag="mask")
        nc.sync.dma_start(mask_sb[:], mask)
        mask_bc = mask_sb[:].unsqueeze(1).unsqueeze(1).broadcast_to([128, 2, 2, 128])

        # Persistent v_aug pair-tiles [batch, s-half, 66]: v at [:, j, i, 0:64],
        # ones at col 64. 4 slots cover the pipeline depth.
        NSLOT = 4
        vaug = []
        for i in range(NSLOT):
            v_t = const.tile([128, 2, 2, 66], BF16, tag=f"vaug{i}")
            nc.gpsimd.memset(v_t[:, :, :, 64:65], 1.0)
            vaug.append(v_t)

        qk_pool = ctx.enter_context(tc.tile_pool(name="qk", bufs=3))
        p_pool = ctx.enter_context(tc.tile_pool(name="pp", bufs=3))
        o_pool = ctx.enter_context(tc.tile_pool(name="oo", bufs=2))
        r_pool = ctx.enter_context(tc.tile_pool(name="rr", bufs=2))
        psqk_pool = ctx.enter_context(tc.tile_pool(name="psqk", bufs=2, space="PSUM"))
        psv_pool = ctx.enter_context(tc.tile_pool(name="psv", bufs=2, space="PSUM"))
        psw_pool = ctx.enter_context(tc.tile_pool(name="psw", bufs=1, space="PSUM"))
        pso_pool = ctx.enter_context(tc.tile_pool(name="pso", bufs=2, space="PSUM"))

        xt_tiles = {}
        st_a = {}  # pair -> (xt, base, ps_qk, q_sb, k_sb) from front_a
        st_b = {}  # pair -> (P, vaug slot) from front_b
        st_o = {}  # group -> out_sb

        def front_a(p):
            """DMA + qk projection + q/k copies for pair p."""
            g, r = divmod(p, 2)
            if r == 0:
                if g == 0:
                    xt_tiles[g] = xt0
                else:
                    xt = xt_pool.tile([128, NCHUNK, 1024], BF16, tag="xt")
                    nc.sync.dma_start(xt[:], xt4[g])
                    xt_tiles[g] = xt
            xt = xt_tiles[g]
            base = 512 * r

            ps_qk = psqk_pool.tile([128, 512], F32, tag="psqk")
            for c in range(NCHUNK):
                nc.tensor.matmul(
                    ps_qk[:],
                    lhsT=wkq_sb[:, c, :],
                    rhs=xt[:, c, base : base + 512],
                    start=(c == 0),
                    stop=(c == NCHUNK - 1),
                )

            k_sb = qk_pool.tile([H, 512], BF16, tag="k")
            nc.vector.tensor_copy(k_sb[:], ps_qk[0:H, :])
            q_sb = qk_pool.tile([H, 512], BF16, tag="q")
            nc.scalar.copy(q_sb[:], ps_qk[H:128, :])

            ps_v = psv_pool.tile([128, 4, H], F32, tag="psv")
            for blk in range(4):
                for c in range(NCHUNK):
                    nc.tensor.matmul(
                        ps_v[:, blk, :],
                        lhsT=xt[:, c, base + 128 * blk : base + 128 * (blk + 1)],
                        rhs=wv_sb[:, c, :],
                        start=(c == 0),
                        stop=(c == NCHUNK - 1),
                    )
            slot = vaug[p % NSLOT]
            nc.scalar.copy(slot[:, 0, :, 0:64], ps_v[:, 0:2, :])
            nc.vector.tensor_copy(slot[:, 1, :, 0:64], ps_v[:, 2:4, :])
            st_a[p] = (q_sb, k_sb, slot)

        def front_b(p):
            """wei matmuls + exp + mask for pair p."""
            q_sb, k_sb, slot = st_a.pop(p)

            psw = psw_pool.tile([128, 1024], F32, tag="psw")
            for j in range(2):
                nc.tensor.matmul(
                    psw[:, 512 * j : 512 * j + 256],
                    lhsT=k_sb[:, 256 * j : 256 * j + 128],
                    rhs=q_sb[:, 256 * j : 256 * j + 256],
                    start=True,
                    stop=True,
                )
                nc.tensor.matmul(
                    psw[:, 512 * j + 256 : 512 * j + 384],
                    lhsT=k_sb[:, 256 * j + 128 : 256 * j + 256],
                    rhs=q_sb[:, 256 * j + 128 : 256 * j + 256],
                    start=True,
                    stop=True,
                )

            P = p_pool.tile([128, 2, 3, 128], BF16, tag="p")
            nc.scalar.activation(
                P[:].rearrange("p j a b -> p (j a b)").rearrange("p (j x) -> p j x", j=2),
                psw[:].rearrange("p (j x) -> p j x", j=2)[:, :, 0:384],
                mybir.ActivationFunctionType.Exp,
                scale=SCALE,
            )
            nc.vector.tensor_mul(P[:, :, 0::2, :], P[:, :, 0::2, :], mask_bc)
            st_b[p] = (P, slot)

        def back(p):
            """Out matmuls + normalize + output DMA for pair p."""
            g, r = divmod(p, 2)
            P, slot = st_b.pop(p)
            pso = pso_pool.tile([128, 4, 65], F32, tag="pso")
            for j in range(2):
                nc.tensor.matmul(
                    pso[:, 2 * j, :],
                    lhsT=P[:, j, 0, :],
                    rhs=slot[:, j, 0, 0:65],
                    start=True,
                    stop=True,
                )
                nc.tensor.matmul(
                    pso[:, 2 * j + 1, :],
                    lhsT=P[:, j, 1, :],
                    rhs=slot[:, j, 0, 0:65],
                    start=True,
                    stop=False,
                )
                nc.tensor.matmul(
                    pso[:, 2 * j + 1, :],
                    lhsT=P[:, j, 2, :],
                    rhs=slot[:, j, 1, 0:65],
                    start=False,
                    stop=True,
                )

            recip = r_pool.tile([128, 4], F32, tag="recip")
            nc.vector.reciprocal(recip[:], pso[:, :, 64])

            if r == 0:
                out_sb = o_pool.tile([128, 8, H], BF16, tag="out")
                st_o[g] = out_sb
            else:
                out_sb = st_o[g]
            nc.vector.tensor_mul(
                out_sb[:, 4 * r : 4 * r + 4, :],
                pso[:, :, 0:64],
                recip[:].unsqueeze(-1).broadcast_to([128, 4, H]),
            )
            if r == 1:
                nc.sync.dma_start(outF[g], st_o.pop(g)[:])

        # Three-stage software pipeline.
        front_a(0)
        front_a(1)
        front_b(0)
        for p in range(2, npair):
            front_a(p)
            front_b(p - 1)
            back(p - 2)
        front_b(npair - 1)
        back(npair - 2)
        back(npair - 1)

    nc.finalize()
    return nc


def _host_inputs(x, Wq, Wk, Wv):
    B_, T_, C_ = x.shape
    assert (B_, T_, C_) == (B, T, C), (B_, T_, C_)
    # xt4[g, p, c, j] = x[4g + j//256, j%256, 128c + p], bf16
    xh = np.ascontiguousarray(
        x.reshape(B // 4, 4, T, NCHUNK, 128).transpose(0, 4, 3, 1, 2)
        .reshape(B // 4, 128, NCHUNK, 4 * T)
    ).astype(BF)
    wkq_h = np.ascontiguousarray(
        np.concatenate([Wk, Wq], axis=1).reshape(NCHUNK, 128, 128).transpose(1, 0, 2)
    ).astype(BF)
    wv_h = np.ascontiguousarray(
        Wv.reshape(NCHUNK, 128, H).transpose(1, 0, 2)
    ).astype(BF)
    mask_h = np.triu(np.ones((128, 128), dtype=np.float32)).astype(BF)
    return xh, wkq_h, wv_h, mask_h


def _host_output(res, bpc=BPC):
    # outF [ngroup, 128, 8, 64] bf16: block 2j+k = batch j of group, t-half k.
    outs = []
    for i in range(N_CORES):
        a = np.asarray(res.results[i]["outF"]).astype(np.float32)
        a = a.reshape(bpc // 4, 128, 4, 2, H).transpose(0, 2, 3, 1, 4)
        outs.append(a.reshape(bpc, T, H))
    return np.ascontiguousarray(np.concatenate(outs, axis=0))


def kernel(x, Wq, Wk, Wv):
    x = np.asarray(x, dtype=np.float32)
    Wq = np.asarray(Wq, dtype=np.float32)
    Wk = np.asarray(Wk, dtype=np.float32)
    Wv = np.asarray(Wv, dtype=np.float32)

    xh, wkq_h, wv_h, mask_h = _host_inputs(x, Wq, Wk, Wv)

    nc = build_nc(BPC)
    in_maps = [
        {
            "xt4": xh[i * NGROUP : (i + 1) * NGROUP],
            "wkq": wkq_h,
            "wv": wv_h,
            "mask": mask_h,
        }
        for i in range(N_CORES)
    ]
    res = run_bass_kernel_spmd(nc, in_maps, list(range(N_CORES)))
    return _host_output(res, BPC)
